# revision 75
# baseline (speedup 1.0000x reference)
"""Trainium2 Bass kernel for nn_BatchPitNorm1d (pairwise Gaussian-CDF KDE + inverse-normal).

Math:  u[b,f] = mean_s Phi((x[b,f] - c[s,f]) / bw[f]),  out = ndtri(u),
       bw = sigmoid(bw_param).

Algorithm: for fixed f, ndtri(u) is a smooth function H_f(x) of x alone, so
instead of B*S*F pairwise Phi evals the kernel:
  1. evaluates the erf-sums g_f(t) at N=8 Chebyshev nodes on a runtime-tight
     domain [-XD, XD] (XD = max|x|), sharded (4 node-groups) x (2 sample
     halves) over 8 cores - NLOC=2 nodes x 1024 samples per core, one fused
     ACT erf instruction per node over the fp8-e4m3 cdf samples (accum_out =
     free-dim sum, per-partition scale/bias precomputed on host),
  2. AllGathers the raw [F, NLOC] blocks (feature-major), reads all 8 blocks
     back with one 3D-AP DMA as [F, 16] and adds the two sample-halves with
     one strided STT,
  3. applies ndtri at the nodes in feature-major [F, N] layout, entirely on
     DVE+ACT with gscale = 2^-12 folded into the coefficients: central
     rational(3,1) in r' = g^2, deg-3 tail polynomial in ln(v') with
     v' = v(1-v) = 0.25 - q^2 (one ACT Ln straight from r'; table load
     hidden under the gather; -sign via ACT Sign), branchless blend via
     copy_predicated,
  4. contracts H against the compile-time inverse-Vandermonde (8 DVE
     scalar_tensor_tensor + accum_out ops) giving per-feature monomial
     coefficients - no PE transpose or matmul anywhere,
  5. evaluates the deg-7 interpolant with a depth-4 Estrin scheme in
     x~ = x/XD (dual-pointer tensor_scalar for c_{2i+1} x~ + c_{2i}, two of
     the four on ACT; x~^2/x~^4/x~^6 precomputed under the gather).

Host-side prep (cheap [F]-sized math): transpose/shard, x~ = x/XD, bw ->
erf scale/bias vectors, cdf -> fp8.  Total error vs the f32 reference:
rel ~4.2e-3 (gate 2e-2).  TimelineSim: 16987 ns (baseline 35949 ns).
"""

import math
from contextlib import ExitStack

import numpy as np

import concourse.bass as bass
import concourse.bacc as bacc
import concourse.tile as tile
from concourse import mybir
from concourse import bass_utils

F32 = mybir.dt.float32
ADD = mybir.AluOpType.add
MUL = mybir.AluOpType.mult

N_CORES = 8
B, S, F = 512, 2048, 128
BL = B // N_CORES          # 64 batch rows per core
N_CHEB = 8                 # Chebyshev nodes / polynomial order
NGRP = 4                   # node groups (cores 2g, 2g+1 share a node group)
NSPL = 2                   # sample splits (even core: half 0, odd: half 1)
NLOC = N_CHEB // NGRP      # 2 nodes per core
SL = S // NSPL             # 1024 samples per core
J = N_CHEB // 2            # even/odd coefficient count

GSCALE = 1.0 / (2.0 * S)   # = 2^-12, exact in f32
PLOW = 0.02425             # central/tail blend point (on v = min(u,1-u))

# Central branch: ndtri(0.5+q) = q*N(r)/D(r), r = q^2, rational (3,1)
# fitted offline for v >= PLOW (max rel err 7e-3 -> ~4e-4 in the final
# interpolant, below the N=8 truncation error).  Coeffs high -> low.
CEN_NUM = [-6.878894024918195, -2.968123707857649, -5.977658457447891,
           2.506654936856209]
CEN_DEN = [-3.429134374657712, 1.0]

# Tail branch: ndtri(v) = P(ln v'), v' = v(1-v) = 0.25 - q^2, fitted on
# v in [1.5e-6, 0.0295] (deg 3, max abs err ~8e-3; node-error sensitivity of
# the final interpolant is ~0.03 rel per unit, so this contributes ~2e-4).
# v' comes straight from r2 = g^2 via one ACT Ln (no Abs needed).
# Coefficients high -> low for the (acc+c)*L Horner form.
TAIL_HL = [0.0007731696376510354, 0.02997354503403231, 0.6047664607065989,
           -0.09085761421359864]
# central mask: v >= PLOW  <=>  r2 <= ((0.25 - PLOW(1-PLOW)) / gscale^2)
MC_R2_THRESH = (0.25 - PLOW * (1 - PLOW)) / GSCALE ** 2


def _cheb_theta():
    return (np.arange(N_CHEB) + 0.5) * np.pi / N_CHEB


def _pq_matrix():
    """Cmono[r, n]: maps H at the nodes to per-feature monomial coefficients.

    y(x~) = sum_r coef_r * x~^r (deg N-1) with coef_r[f] = sum_n h[f, n] *
    Cmono[r, n]; Cmono = inv(Vandermonde(normalized nodes)) folds the whole
    interpolation into one compile-time constant, so the device needs no PE
    transpose/matmul at all.  XD-independent.
    """
    xt = np.cos(_cheb_theta())
    V = np.vander(xt, N_CHEB, increasing=True)    # V[n, r] = xt_n^r
    return np.ascontiguousarray(np.linalg.inv(V)).astype(np.float32)


def build(with_collective=True, debug_taps=False):
    nc = bacc.Bacc("TRN2", target_bir_lowering=False, debug=False,
                   enable_asserts=False, num_devices=N_CORES)

    # Inputs arrive pre-transposed (feature-major) from the host shard step.
    xw = nc.dram_tensor("xw", [F, BL], F32, kind="ExternalInput")       # x~ = x/XD
    # cdf in fp8-e4m3: quarters the input DMA transfer time; erf-argument
    # rounding error mostly averages out over the 2048-sample mean
    # (validated: rel 3.8e-3 at N=8 vs 3.6e-3 in f32).
    cdf_t = nc.dram_tensor("cdf_t", [F, SL], mybir.dt.float8e4,
                           kind="ExternalInput")
    consts = nc.dram_tensor("consts", [F, 1 + NLOC], F32, kind="ExternalInput")  # -a | a*t_j
    out = nc.dram_tensor("out", [F, BL], F32, kind="ExternalOutput")
    taps = {}
    if debug_taps:
        for nm, shp in [("d_gacc", [F, NLOC]), ("d_gsum", [F, N_CHEB]),
                        ("d_h", [F, N_CHEB]), ("d_coef", [F, N_CHEB])]:
            taps[nm] = nc.dram_tensor(nm, shp, F32, kind="ExternalOutput")

    pq_h = nc.inline_tensor(_pq_matrix().reshape(1, N_CHEB * N_CHEB), name="pq")

    with tile.TileContext(nc) as tc, ExitStack() as ctx:
        sb = ctx.enter_context(tc.tile_pool(name="sb", bufs=1))
        psum = ctx.enter_context(tc.tile_pool(name="psum", bufs=1, space="PSUM"))
        dram = ctx.enter_context(tc.tile_pool(name="dram", bufs=1, space="DRAM"))

        D = nc.vector    # DVE
        P = nc.gpsimd    # Pool
        A = nc.scalar    # ACT
        SP = nc.sync     # SP

        def ts(eng, name, in0, s1, s2=None, op0=MUL, op1=ADD, w=N_CHEB):
            t = sb.tile([F, w], F32, name=name, tag=name)
            if s2 is None:
                eng.tensor_scalar(out=t, in0=in0, scalar1=s1, scalar2=None, op0=op0)
            else:
                eng.tensor_scalar(out=t, in0=in0, scalar1=s1, scalar2=s2,
                                  op0=op0, op1=op1)
            return t

        def stt(eng, name, in0, s, in1, op0=ADD, op1=MUL, w=N_CHEB, out=None):
            t = out if out is not None else sb.tile([F, w], F32, name=name, tag=name)
            eng.scalar_tensor_tensor(out=t, in0=in0, scalar=s, in1=in1,
                                     op0=op0, op1=op1)
            return t

        # ---------------- input DMAs (one per queue, issued up front)
        cT = sb.tile([F, SL], mybir.dt.float8e4, name="cT")
        SP.dma_start(out=cT, in_=cdf_t[:, :])
        cst = sb.tile([F, 1 + NLOC], F32, name="cst")
        P.dma_start(out=cst, in_=consts[:, :])
        xw_sb = sb.tile([F, BL], F32, name="xw")
        P.dma_start(out=xw_sb, in_=xw[:, :])
        # PQ constants replicated to every partition (DMA broadcast read)
        pq_sb = sb.tile([F, N_CHEB * N_CHEB], F32, name="pq")
        P.dma_start(out=pq_sb, in_=bass.AP(tensor=pq_h, offset=0,
                                           ap=[[0, F], [1, N_CHEB * N_CHEB]]))

        xt = xw_sb[:, :BL]

        half_c = sb.tile([F, 1], F32, name="halfc")
        D.memset(half_c, 0.5)
        quart_c = sb.tile([F, 1], F32, name="quartc")
        D.memset(quart_c, 0.25)
        # Data-independent dummy erf: forces the erf table load at t~1us,
        # while the cdf DMA is still in flight (instead of right before erf0).
        erfdum = sb.tile([F, 1], F32, name="erfdum")
        A.activation(out=erfdum, in_=half_c,
                     func=mybir.ActivationFunctionType.Erf, scale=0.0,
                     bias=half_c[:, 0:1])


        # ---------------- x~ powers for the Estrin evaluation (hidden under
        # the grid + gather phases; only x2/x4/x6 are needed)
        x2 = stt(D, "x2", xt, 0.0, xt, w=BL)
        x4 = stt(D, "x4", x2, 0.0, x2, w=BL)
        x6 = stt(D, "x6", x2, 0.0, x4, w=BL)

        # ---------------- grid: gacc[f, j] = sum_s erf(-a_f*c_sf + a_f*t_j)
        gacc = sb.tile([F, NLOC], F32, name="gacc")
        scr = psum.tile([F, SL], F32, name="scr", tag="scr")
        for j in range(NLOC):
            A.activation(out=scr, in_=cT, func=mybir.ActivationFunctionType.Erf,
                         bias=cst[:, 1 + j:2 + j], scale=cst[:, 0:1],
                         accum_out=gacc[:, j:j + 1])
        # Force the Ln table switch right after the grid so the ~1.3us load
        # hides under the gather round-trip.  Reads the last accum column so
        # the scheduler cannot hoist it between the erfs (which would force
        # extra erf-table reloads).
        lndum = sb.tile([F, 1], F32, name="lndum")
        A.activation(out=lndum, in_=gacc[:, NLOC - 1:NLOC],
                     func=mybir.ActivationFunctionType.Ln, scale=0.0,
                     bias=half_c[:, 0:1])

        # ---------------- exchange: AllGather of the [F, NLOC] blocks
        cin = dram.tile([F, NLOC], F32, tag="cin")
        SP.dma_start(out=cin[:, :], in_=gacc)
        cout = dram.tile([N_CORES, F, NLOC], F32, tag="cout",
                         addr_space="Shared" if with_collective else "Local")
        if with_collective:
            P.collective_compute(
                "AllGather", mybir.AluOpType.bypass,
                replica_groups=[list(range(N_CORES))],
                ins=[cin.opt()], outs=[cout.opt()],
            )
        # Single readback of all 8 [F, NLOC] blocks, rank-major:
        # gbig[f, rank*NLOC + j] = cout[rank][f][j], rank = g*NSPL + h.
        gbig = sb.tile([F, N_CORES * NLOC], F32, name="gbig")
        if with_collective:
            src_ap = bass.AP(
                tensor=cout.tensor, offset=cout.offset,
                ap=[[NLOC, F], [F * NLOC, N_CORES], [1, NLOC]])
        else:  # stand-in: broadcast-read own block (timing model only)
            src_ap = bass.AP(
                tensor=cin.tensor, offset=cin.offset,
                ap=[[NLOC, F], [0, N_CORES], [1, NLOC]])
        SP.dma_start(out=gbig[:, :], in_=src_ap)

        # g_sum[f, g*NLOC+j] = sum_h gbig[f, (g*NSPL+h)*NLOC + j]
        g_sum = sb.tile([F, N_CHEB], F32, name="gsum")
        gb_w = N_CORES * NLOC
        h0_ap = bass.AP(tensor=gbig.tensor, offset=gbig.offset,
                        ap=[[gb_w, F], [NSPL * NLOC, NGRP], [1, NLOC]])
        h1_ap = bass.AP(tensor=gbig.tensor, offset=gbig.offset + NLOC,
                        ap=[[gb_w, F], [NSPL * NLOC, NGRP], [1, NLOC]])
        D.scalar_tensor_tensor(out=g_sum, in0=h0_ap, scalar=0.0, in1=h1_ap,
                               op0=ADD, op1=ADD)

        # ---------------- ndtri at the nodes, feature-major [F, N]
        # gscale = 1/(2S) = 2^-12 is an exact power of two, so it is folded
        # into the rational coefficients (exact f32 scaling): work directly on
        # r' = g^2 and finish with *g instead of computing q = g*gscale.
        CN = [CEN_NUM[i] * GSCALE ** (2 * (3 - i) + 1) for i in range(4)]
        CD = [CEN_DEN[i] * GSCALE ** (2 * (1 - i)) for i in range(2)]
        r2 = stt(D, "r2", g_sum, 0.0, g_sum)
        mc = sb.tile([F, N_CHEB], mybir.dt.uint8, name="mc")
        D.tensor_scalar(out=mc, in0=r2, scalar1=float(MC_R2_THRESH),
                        scalar2=None, op0=mybir.AluOpType.is_le)
        # ACT: lnv' = Ln(0.25 - r2*gscale^2); v' = v(1-v) stays >= ~5e-6 for
        # this data (empirical node minimum).
        lnv = sb.tile([F, N_CHEB], F32, name="lnv")
        A.activation(out=lnv, in_=r2, func=mybir.ActivationFunctionType.Ln,
                     scale=-GSCALE * GSCALE, bias=quart_c[:, 0:1])
        # central: q*N(r)/D(r) in the scaled variables
        ca = ts(D, "ca0", r2, float(CN[0]))
        ca = stt(D, "ca1", ca, float(CN[1]), r2)
        ca = stt(D, "ca2", ca, float(CN[2]), r2)
        nq = stt(D, "nq", ca, float(CN[3]), g_sum)
        df = ts(D, "df", r2, float(CD[0]), float(CD[1]), op0=MUL, op1=ADD)
        rec = sb.tile([F, N_CHEB], F32, name="rec")
        D.reciprocal(out=rec, in_=df)
        xc = stt(D, "xc", nq, 0.0, rec)
        # tail: P(ln v) * (-sign(g)); Sign is in every ACT table set
        nsgn = sb.tile([F, N_CHEB], F32, name="nsgn")
        A.activation(out=nsgn, in_=g_sum,
                     func=mybir.ActivationFunctionType.Sign, scale=-1.0)
        ta = ts(D, "ta0", lnv, float(TAIL_HL[0]))
        for i, c in enumerate(TAIL_HL[1:-1]):
            ta = stt(D, f"ta{i + 1}", ta, float(c), lnv)
        h = sb.tile([F, N_CHEB], F32, name="h")
        stt(D, "tsgn", ta, float(TAIL_HL[-1]), nsgn, out=h)
        # blend: overwrite central region with xc
        D.copy_predicated(h, mc, xc)

        # ---------------- fit: monomial coefficients straight from h.
        # coef[:, r] = sum_n h[:, n] * Cmono[r, n] via 8 independent DVE
        # scalar_tensor_tensor + accum_out ops (accum = free-dim sum).
        coef = sb.tile([F, N_CHEB], F32, name="coef")
        for r in range(N_CHEB):
            ttr_scr = sb.tile([F, N_CHEB], F32, name=f"ttrs{r}")
            D.scalar_tensor_tensor(
                out=ttr_scr, in0=h, scalar=0.0,
                in1=pq_sb[:, r * N_CHEB:(r + 1) * N_CHEB],
                op0=ADD, op1=MUL, accum_out=coef[:, r:r + 1])

        # ---------------- evaluate: deg-7 Estrin in x~, depth 4.
        # g_i = c_{2i+1}*x~ + c_{2i} (dual-pointer tensor_scalar), then
        # y = g0 + g1*x2 + g2*x4 + g3*x6 with the powers precomputed above.
        gs_ = []
        for i in range(4):
            g_t = sb.tile([F, BL], F32, name=f"ge{i}")
            if i % 2 == 1:  # offload half the g_i to the idle ACT engine
                A.activation(out=g_t, in_=xt,
                             func=mybir.ActivationFunctionType.Identity,
                             scale=coef[:, 2 * i + 1:2 * i + 2],
                             bias=coef[:, 2 * i:2 * i + 1])
            else:
                D.tensor_scalar(out=g_t, in0=xt,
                                scalar1=coef[:, 2 * i + 1:2 * i + 2],
                                scalar2=coef[:, 2 * i:2 * i + 1],
                                op0=MUL, op1=ADD)
            gs_.append(g_t)
        m1 = stt(D, "m1", gs_[1], 0.0, x2, w=BL)
        m2 = stt(D, "m2", gs_[2], 0.0, x4, w=BL)
        m3 = stt(D, "m3", gs_[3], 0.0, x6, w=BL)
        s1 = stt(D, "s1", gs_[0], 0.0, m1, op1=ADD, w=BL)
        s2 = stt(D, "s2", m2, 0.0, m3, op1=ADD, w=BL)
        y = stt(D, "y", s1, 0.0, s2, op1=ADD, w=BL)

        SP.dma_start(out=out[:, :], in_=y)

        if debug_taps:
            for nm, t in [("d_gacc", gacc), ("d_gsum", g_sum), ("d_h", h),
                          ("d_coef", coef)]:
                SP.dma_start(out=taps[nm][:, :], in_=t)

    nc.compile()
    return nc


_CACHE = {}


def _get_nc():
    if "nc" not in _CACHE:
        _CACHE["nc"] = build(with_collective=True)
    return _CACHE["nc"]


def kernel(x, cdf_data, bw_param):
    x = np.ascontiguousarray(x, dtype=np.float32)
    cdf_data = np.ascontiguousarray(cdf_data, dtype=np.float32)
    bw_param = np.ascontiguousarray(bw_param, dtype=np.float32)
    nc = _get_nc()

    xd = float(np.abs(x).max()) * 1.0005
    th = _cheb_theta()
    t_nodes = (xd * np.cos(th)).astype(np.float32)              # [N]
    bw = (1.0 / (1.0 + np.exp(-bw_param.astype(np.float64))))[0]
    a = (1.0 / (bw * math.sqrt(2.0))).astype(np.float32)        # [F]

    import ml_dtypes
    xt = np.clip(x.T, -xd, xd).astype(np.float32) / np.float32(xd)   # [F, B]
    cdf_halves = [np.ascontiguousarray(
                      cdf_data[h * SL:(h + 1) * SL].T.astype(ml_dtypes.float8_e4m3))
                  for h in range(NSPL)]                          # each [F, SL]

    in_maps = []
    for i in range(N_CORES):
        g, h = i // NSPL, i % NSPL
        bias = a[:, None] * t_nodes[None, g * NLOC:(g + 1) * NLOC]  # [F, NLOC]
        consts_i = np.concatenate([-a[:, None], bias], axis=1)
        in_maps.append({
            "xw": np.ascontiguousarray(xt[:, i * BL:(i + 1) * BL]),
            "cdf_t": cdf_halves[h],
            "consts": np.ascontiguousarray(consts_i.astype(np.float32)),
        })
    res = bass_utils.run_bass_kernel_spmd(nc, in_maps, core_ids=list(range(N_CORES)))
    return np.concatenate([res.results[i]["out"].T for i in range(N_CORES)], axis=0)



# revision 77
# speedup vs baseline: 1.0135x; 1.0135x over previous
"""Trainium2 Bass kernel for nn_BatchPitNorm1d (pairwise Gaussian-CDF KDE + inverse-normal).

Math:  u[b,f] = mean_s Phi((x[b,f] - c[s,f]) / bw[f]),  out = ndtri(u),
       bw = sigmoid(bw_param).

Algorithm: for fixed f, ndtri(u) is a smooth function H_f(x) of x alone, so
instead of B*S*F pairwise Phi evals the kernel:
  1. evaluates the erf-sums g_f(t) at N=8 Chebyshev nodes on a runtime-tight
     domain [-XD, XD] (XD = max|x|), sharded (4 node-groups) x (2 sample
     halves) over 8 cores - NLOC=2 nodes x 1024 samples per core, one fused
     ACT erf instruction per node over the fp8-e4m3 cdf samples (accum_out =
     free-dim sum, per-partition scale/bias precomputed on host),
  2. AllGathers the raw [F, NLOC] blocks (feature-major), reads all 8 blocks
     back with one 3D-AP DMA as [F, 16] and adds the two sample-halves with
     one strided STT,
  3. applies ndtri at the nodes in feature-major [F, N] layout, entirely on
     DVE+ACT with gscale = 2^-12 folded into the coefficients: central
     rational(3,1) in r' = g^2, deg-3 tail polynomial in ln(v') with
     v' = v(1-v) = 0.25 - q^2 (one ACT Ln straight from r'; table load
     hidden under the gather; -sign via ACT Sign), branchless blend via
     copy_predicated,
  4. contracts H against the compile-time inverse-Vandermonde (8 DVE
     scalar_tensor_tensor + accum_out ops) giving per-feature monomial
     coefficients - no PE transpose or matmul anywhere,
  5. evaluates the deg-7 interpolant with a depth-4 Estrin scheme in
     x~ = x/XD (dual-pointer tensor_scalar for c_{2i+1} x~ + c_{2i}, two of
     the four on ACT; x~^2/x~^4/x~^6 precomputed under the gather).

Host-side prep (cheap [F]-sized math): transpose/shard, x~ = x/XD, bw ->
erf scale/bias vectors, cdf -> fp8.  Total error vs the f32 reference:
rel ~4.2e-3 (gate 2e-2).  TimelineSim: 16987 ns (baseline 35949 ns).
"""

import math
from contextlib import ExitStack

import numpy as np

import concourse.bass as bass
import concourse.bacc as bacc
import concourse.tile as tile
from concourse import mybir
from concourse import bass_utils

F32 = mybir.dt.float32
ADD = mybir.AluOpType.add
MUL = mybir.AluOpType.mult

N_CORES = 8
B, S, F = 512, 2048, 128
BL = B // N_CORES          # 64 batch rows per core
N_CHEB = 8                 # Chebyshev nodes / polynomial order
NGRP = 8                   # node groups (one per core)
NSPL = 1                   # no sample split: each core does full S for its node
NLOC = N_CHEB // NGRP      # 1 node per core
SL = S // NSPL             # 1024 samples per core
J = N_CHEB // 2            # even/odd coefficient count

GSCALE = 1.0 / (2.0 * S)   # = 2^-12, exact in f32
PLOW = 0.02425             # central/tail blend point (on v = min(u,1-u))

# Central branch: ndtri(0.5+q) = q*N(r)/D(r), r = q^2, rational (3,1)
# fitted offline for v >= PLOW (max rel err 7e-3 -> ~4e-4 in the final
# interpolant, below the N=8 truncation error).  Coeffs high -> low.
CEN_NUM = [-6.878894024918195, -2.968123707857649, -5.977658457447891,
           2.506654936856209]
CEN_DEN = [-3.429134374657712, 1.0]

# Tail branch: ndtri(v) = P(ln v'), v' = v(1-v) = 0.25 - q^2, fitted on
# v in [1.5e-6, 0.0295] (deg 3, max abs err ~8e-3; node-error sensitivity of
# the final interpolant is ~0.03 rel per unit, so this contributes ~2e-4).
# v' comes straight from r2 = g^2 via one ACT Ln (no Abs needed).
# Coefficients high -> low for the (acc+c)*L Horner form.
TAIL_HL = [0.0007731696376510354, 0.02997354503403231, 0.6047664607065989,
           -0.09085761421359864]
# central mask: v >= PLOW  <=>  r2 <= ((0.25 - PLOW(1-PLOW)) / gscale^2)
MC_R2_THRESH = (0.25 - PLOW * (1 - PLOW)) / GSCALE ** 2


def _cheb_theta():
    return (np.arange(N_CHEB) + 0.5) * np.pi / N_CHEB


def _pq_matrix():
    """Cmono[r, n]: maps H at the nodes to per-feature monomial coefficients.

    y(x~) = sum_r coef_r * x~^r (deg N-1) with coef_r[f] = sum_n h[f, n] *
    Cmono[r, n]; Cmono = inv(Vandermonde(normalized nodes)) folds the whole
    interpolation into one compile-time constant, so the device needs no PE
    transpose/matmul at all.  XD-independent.
    """
    xt = np.cos(_cheb_theta())
    V = np.vander(xt, N_CHEB, increasing=True)    # V[n, r] = xt_n^r
    return np.ascontiguousarray(np.linalg.inv(V)).astype(np.float32)


def build(with_collective=True, debug_taps=False):
    nc = bacc.Bacc("TRN2", target_bir_lowering=False, debug=False,
                   enable_asserts=False, num_devices=N_CORES)

    # Inputs arrive pre-transposed (feature-major) from the host shard step.
    xw = nc.dram_tensor("xw", [F, BL], F32, kind="ExternalInput")       # x~ = x/XD
    # cdf in fp8-e4m3: quarters the input DMA transfer time; erf-argument
    # rounding error mostly averages out over the 2048-sample mean
    # (validated: rel 3.8e-3 at N=8 vs 3.6e-3 in f32).
    cdf_t = nc.dram_tensor("cdf_t", [F, SL], mybir.dt.float8e4,
                           kind="ExternalInput")
    consts = nc.dram_tensor("consts", [F, 1 + NLOC], F32, kind="ExternalInput")  # -a | a*t_j
    out = nc.dram_tensor("out", [F, BL], F32, kind="ExternalOutput")
    taps = {}
    if debug_taps:
        for nm, shp in [("d_gacc", [F, NLOC]), ("d_gsum", [F, N_CHEB]),
                        ("d_h", [F, N_CHEB]), ("d_coef", [F, N_CHEB])]:
            taps[nm] = nc.dram_tensor(nm, shp, F32, kind="ExternalOutput")

    pq_h = nc.inline_tensor(_pq_matrix().reshape(1, N_CHEB * N_CHEB), name="pq")

    with tile.TileContext(nc) as tc, ExitStack() as ctx:
        sb = ctx.enter_context(tc.tile_pool(name="sb", bufs=1))
        psum = ctx.enter_context(tc.tile_pool(name="psum", bufs=1, space="PSUM"))
        dram = ctx.enter_context(tc.tile_pool(name="dram", bufs=1, space="DRAM"))

        D = nc.vector    # DVE
        P = nc.gpsimd    # Pool
        A = nc.scalar    # ACT
        SP = nc.sync     # SP

        def ts(eng, name, in0, s1, s2=None, op0=MUL, op1=ADD, w=N_CHEB):
            t = sb.tile([F, w], F32, name=name, tag=name)
            if s2 is None:
                eng.tensor_scalar(out=t, in0=in0, scalar1=s1, scalar2=None, op0=op0)
            else:
                eng.tensor_scalar(out=t, in0=in0, scalar1=s1, scalar2=s2,
                                  op0=op0, op1=op1)
            return t

        def stt(eng, name, in0, s, in1, op0=ADD, op1=MUL, w=N_CHEB, out=None):
            t = out if out is not None else sb.tile([F, w], F32, name=name, tag=name)
            eng.scalar_tensor_tensor(out=t, in0=in0, scalar=s, in1=in1,
                                     op0=op0, op1=op1)
            return t

        # ---------------- input DMAs (one per queue, issued up front)
        cT = sb.tile([F, SL], mybir.dt.float8e4, name="cT")
        SP.dma_start(out=cT, in_=cdf_t[:, :])
        cst = sb.tile([F, 1 + NLOC], F32, name="cst")
        P.dma_start(out=cst, in_=consts[:, :])
        xw_sb = sb.tile([F, BL], F32, name="xw")
        P.dma_start(out=xw_sb, in_=xw[:, :])
        # PQ constants replicated to every partition (DMA broadcast read)
        pq_sb = sb.tile([F, N_CHEB * N_CHEB], F32, name="pq")
        P.dma_start(out=pq_sb, in_=bass.AP(tensor=pq_h, offset=0,
                                           ap=[[0, F], [1, N_CHEB * N_CHEB]]))

        xt = xw_sb[:, :BL]

        half_c = sb.tile([F, 1], F32, name="halfc")
        D.memset(half_c, 0.5)
        quart_c = sb.tile([F, 1], F32, name="quartc")
        D.memset(quart_c, 0.25)
        # Data-independent dummy erf: forces the erf table load at t~1us,
        # while the cdf DMA is still in flight (instead of right before erf0).
        erfdum = sb.tile([F, 1], F32, name="erfdum")
        A.activation(out=erfdum, in_=half_c,
                     func=mybir.ActivationFunctionType.Erf, scale=0.0,
                     bias=half_c[:, 0:1])


        # ---------------- x~ powers for the Estrin evaluation (hidden under
        # the grid + gather phases; only x2/x4/x6 are needed)
        x2 = stt(D, "x2", xt, 0.0, xt, w=BL)
        x4 = stt(D, "x4", x2, 0.0, x2, w=BL)
        x6 = stt(D, "x6", x2, 0.0, x4, w=BL)

        # ---------------- grid: gacc[f, j] = sum_s erf(-a_f*c_sf + a_f*t_j)
        gacc = sb.tile([F, NLOC], F32, name="gacc")
        scr = psum.tile([F, SL], F32, name="scr", tag="scr")
        for j in range(NLOC):
            A.activation(out=scr, in_=cT, func=mybir.ActivationFunctionType.Erf,
                         bias=cst[:, 1 + j:2 + j], scale=cst[:, 0:1],
                         accum_out=gacc[:, j:j + 1])
        # Force the Ln table switch right after the grid so the ~1.3us load
        # hides under the gather round-trip.  Reads the last accum column so
        # the scheduler cannot hoist it between the erfs (which would force
        # extra erf-table reloads).
        lndum = sb.tile([F, 1], F32, name="lndum")
        A.activation(out=lndum, in_=gacc[:, NLOC - 1:NLOC],
                     func=mybir.ActivationFunctionType.Ln, scale=0.0,
                     bias=half_c[:, 0:1])

        # ---------------- exchange: AllGather of the [F, NLOC] blocks
        cin = dram.tile([F, NLOC], F32, tag="cin")
        SP.dma_start(out=cin[:, :], in_=gacc)
        cout = dram.tile([N_CORES, F, NLOC], F32, tag="cout",
                         addr_space="Shared" if with_collective else "Local")
        if with_collective:
            P.collective_compute(
                "AllGather", mybir.AluOpType.bypass,
                replica_groups=[list(range(N_CORES))],
                ins=[cin.opt()], outs=[cout.opt()],
            )
        # Single readback of all 8 [F, NLOC] blocks, rank-major:
        # gbig[f, rank*NLOC + j] = cout[rank][f][j], rank = g*NSPL + h.
        gbig = sb.tile([F, N_CORES * NLOC], F32, name="gbig")
        if with_collective:
            src_ap = bass.AP(
                tensor=cout.tensor, offset=cout.offset,
                ap=[[NLOC, F], [F * NLOC, N_CORES], [1, NLOC]])
        else:  # stand-in: broadcast-read own block (timing model only)
            src_ap = bass.AP(
                tensor=cin.tensor, offset=cin.offset,
                ap=[[NLOC, F], [0, N_CORES], [1, NLOC]])
        SP.dma_start(out=gbig[:, :], in_=src_ap)

        # NSPL=1: rank == node index, so the gathered tile IS the full sums.
        g_sum = gbig

        # ---------------- ndtri at the nodes, feature-major [F, N]
        # gscale = 1/(2S) = 2^-12 is an exact power of two, so it is folded
        # into the rational coefficients (exact f32 scaling): work directly on
        # r' = g^2 and finish with *g instead of computing q = g*gscale.
        CN = [CEN_NUM[i] * GSCALE ** (2 * (3 - i) + 1) for i in range(4)]
        CD = [CEN_DEN[i] * GSCALE ** (2 * (1 - i)) for i in range(2)]
        r2 = stt(D, "r2", g_sum, 0.0, g_sum)
        mc = sb.tile([F, N_CHEB], mybir.dt.uint8, name="mc")
        D.tensor_scalar(out=mc, in0=r2, scalar1=float(MC_R2_THRESH),
                        scalar2=None, op0=mybir.AluOpType.is_le)
        # ACT: lnv' = Ln(0.25 - r2*gscale^2); v' = v(1-v) stays >= ~5e-6 for
        # this data (empirical node minimum).
        lnv = sb.tile([F, N_CHEB], F32, name="lnv")
        A.activation(out=lnv, in_=r2, func=mybir.ActivationFunctionType.Ln,
                     scale=-GSCALE * GSCALE, bias=quart_c[:, 0:1])
        # central: q*N(r)/D(r) in the scaled variables
        ca = ts(D, "ca0", r2, float(CN[0]))
        ca = stt(D, "ca1", ca, float(CN[1]), r2)
        ca = stt(D, "ca2", ca, float(CN[2]), r2)
        nq = stt(D, "nq", ca, float(CN[3]), g_sum)
        df = ts(D, "df", r2, float(CD[0]), float(CD[1]), op0=MUL, op1=ADD)
        rec = sb.tile([F, N_CHEB], F32, name="rec")
        D.reciprocal(out=rec, in_=df)
        xc = stt(D, "xc", nq, 0.0, rec)
        # tail: P(ln v) * (-sign(g)); Sign is in every ACT table set
        nsgn = sb.tile([F, N_CHEB], F32, name="nsgn")
        A.activation(out=nsgn, in_=g_sum,
                     func=mybir.ActivationFunctionType.Sign, scale=-1.0)
        ta = ts(D, "ta0", lnv, float(TAIL_HL[0]))
        for i, c in enumerate(TAIL_HL[1:-1]):
            ta = stt(D, f"ta{i + 1}", ta, float(c), lnv)
        h = sb.tile([F, N_CHEB], F32, name="h")
        stt(D, "tsgn", ta, float(TAIL_HL[-1]), nsgn, out=h)
        # blend: overwrite central region with xc
        D.copy_predicated(h, mc, xc)

        # ---------------- fit: monomial coefficients straight from h.
        # coef[:, r] = sum_n h[:, n] * Cmono[r, n] via 8 independent DVE
        # scalar_tensor_tensor + accum_out ops (accum = free-dim sum).
        coef = sb.tile([F, N_CHEB], F32, name="coef")
        for r in range(N_CHEB):
            ttr_scr = sb.tile([F, N_CHEB], F32, name=f"ttrs{r}")
            D.scalar_tensor_tensor(
                out=ttr_scr, in0=h, scalar=0.0,
                in1=pq_sb[:, r * N_CHEB:(r + 1) * N_CHEB],
                op0=ADD, op1=MUL, accum_out=coef[:, r:r + 1])

        # ---------------- evaluate: deg-7 Estrin in x~, depth 4.
        # g_i = c_{2i+1}*x~ + c_{2i} (dual-pointer tensor_scalar), then
        # y = g0 + g1*x2 + g2*x4 + g3*x6 with the powers precomputed above.
        gs_ = []
        for i in range(4):
            g_t = sb.tile([F, BL], F32, name=f"ge{i}")
            if i % 2 == 1:  # offload half the g_i to the idle ACT engine
                A.activation(out=g_t, in_=xt,
                             func=mybir.ActivationFunctionType.Identity,
                             scale=coef[:, 2 * i + 1:2 * i + 2],
                             bias=coef[:, 2 * i:2 * i + 1])
            else:
                D.tensor_scalar(out=g_t, in0=xt,
                                scalar1=coef[:, 2 * i + 1:2 * i + 2],
                                scalar2=coef[:, 2 * i:2 * i + 1],
                                op0=MUL, op1=ADD)
            gs_.append(g_t)
        m1 = stt(D, "m1", gs_[1], 0.0, x2, w=BL)
        m2 = stt(D, "m2", gs_[2], 0.0, x4, w=BL)
        m3 = stt(D, "m3", gs_[3], 0.0, x6, w=BL)
        s1 = stt(D, "s1", gs_[0], 0.0, m1, op1=ADD, w=BL)
        s2 = stt(D, "s2", m2, 0.0, m3, op1=ADD, w=BL)
        y = stt(D, "y", s1, 0.0, s2, op1=ADD, w=BL)

        SP.dma_start(out=out[:, :], in_=y)

        if debug_taps:
            for nm, t in [("d_gacc", gacc), ("d_gsum", g_sum), ("d_h", h),
                          ("d_coef", coef)]:
                SP.dma_start(out=taps[nm][:, :], in_=t)

    nc.compile()
    return nc


_CACHE = {}


def _get_nc():
    if "nc" not in _CACHE:
        _CACHE["nc"] = build(with_collective=True)
    return _CACHE["nc"]


def kernel(x, cdf_data, bw_param):
    x = np.ascontiguousarray(x, dtype=np.float32)
    cdf_data = np.ascontiguousarray(cdf_data, dtype=np.float32)
    bw_param = np.ascontiguousarray(bw_param, dtype=np.float32)
    nc = _get_nc()

    xd = float(np.abs(x).max()) * 1.0005
    th = _cheb_theta()
    t_nodes = (xd * np.cos(th)).astype(np.float32)              # [N]
    bw = (1.0 / (1.0 + np.exp(-bw_param.astype(np.float64))))[0]
    a = (1.0 / (bw * math.sqrt(2.0))).astype(np.float32)        # [F]

    import ml_dtypes
    xt = np.clip(x.T, -xd, xd).astype(np.float32) / np.float32(xd)   # [F, B]
    cdf_halves = [np.ascontiguousarray(
                      cdf_data[h * SL:(h + 1) * SL].T.astype(ml_dtypes.float8_e4m3))
                  for h in range(NSPL)]                          # each [F, SL]

    in_maps = []
    for i in range(N_CORES):
        g, h = i // NSPL, i % NSPL
        bias = a[:, None] * t_nodes[None, g * NLOC:(g + 1) * NLOC]  # [F, NLOC]
        consts_i = np.concatenate([-a[:, None], bias], axis=1)
        in_maps.append({
            "xw": np.ascontiguousarray(xt[:, i * BL:(i + 1) * BL]),
            "cdf_t": cdf_halves[h],
            "consts": np.ascontiguousarray(consts_i.astype(np.float32)),
        })
    res = bass_utils.run_bass_kernel_spmd(nc, in_maps, core_ids=list(range(N_CORES)))
    return np.concatenate([res.results[i]["out"].T for i in range(N_CORES)], axis=0)

